# revision 1
# baseline (speedup 1.0000x reference)
"""2-layer GCN (PyG GCNConv x2 + log_softmax) on 8 Trainium2 NeuronCores.

Strategy:
  - Nodes are permuted and sharded across 8 cores (12544 slots/core, 98 groups
    of 128). A per-core vector bin-packing balances, for every (group, source
    block) cell, the number of in-edges, so the static chunk layout (shared by
    all SPMD cores) wastes only ~2% padding.
  - Aggregation A_norm @ Z is commuted past the dense transforms:
        out_i = dinv_i * ((sum_{j->i} z'_j + z'_i) @ W) + b,   z' = dinv .* z
    so each layer is: gather z' rows by edge source (dma_gather, bf16 table),
    segment-sum per 128-node destination group via a 0/1 mask matmul
    accumulated in PSUM (feature-major), add the self row, dense-transform by
    W (fp32), scale+bias+activation, log_softmax at the end.
  - Gather tables are indexed with int16 relative indices inside 4 source
    blocks of 25088 rows (= 2 cores' shards). Both layers reuse the same
    edge layout/indices; layer-2's table (h1') is built on-device and
    exchanged with a single AllGather collective.
"""
import sys
sys.path.insert(0, '/opt/trn_rl_repo')

import numpy as np
import ml_dtypes
from contextlib import ExitStack

from concourse import bass, bacc, mybir, tile
from concourse.bass_utils import run_bass_kernel_spmd
from concourse.masks import make_identity

N = 100000
E = 1600000
DIN, DH, DOUT = 128, 128, 64
NC = 8
CPC = 12500            # real nodes per core
NPC = 12544            # padded slots per core (98 * 128)
G = 98                 # node groups of 128 per core
BS = 25088             # source block size (NPC * NC / 4), int16-addressable
NB = 4                 # source blocks
TIGHT, LOOSE, NSPILL = 512, 640, 6
SETG = 10              # groups per gather-call set

_cache = {}


def _pack_core(dv, caps):
    """Greedy vector bin-packing: nodes (rows of dv, [n,NB]) into G bins of
    128 slots with per-block capacity caps [G,NB]. Returns bin assignment."""
    n = dv.shape[0]
    order = np.argsort(-dv.sum(1), kind='stable')
    rem = caps.astype(np.int64).copy()
    slots = np.full(G, 128, np.int64)
    assign = np.empty(n, np.int32)
    for i in order:
        d = dv[i]
        ok = (slots > 0) & (rem >= d[None, :]).all(1)
        if ok.any():
            cand = np.where(ok)[0]
            sl = (rem[cand] - d[None, :]).min(1)
            j = cand[np.argmax(sl * 1000 + slots[cand])]
        else:
            cand = np.where(slots > 0)[0]
            over = np.maximum(d[None, :] - rem[cand], 0).sum(1)
            j = cand[np.argmin(over)]
        assign[i] = j
        rem[j] -= d
        slots[j] -= 1
    return assign


def _preprocess(x, edge_index, W1, b1, W2, b2):
    src = edge_index[0].astype(np.int64)
    dst = edge_index[1].astype(np.int64)
    indeg = np.bincount(dst, minlength=N)
    dinv = (1.0 / np.sqrt((indeg + 1).astype(np.float64))).astype(np.float32)

    core_of = np.minimum(np.arange(N) // CPC, NC - 1)
    blk_of = core_of // 2                       # source block = core pair
    sblk = blk_of[src]

    # demand vector: per node, in-edges split by source block
    dvec = np.zeros((N, NB), np.int32)
    for b in range(NB):
        dvec[:, b] = np.bincount(dst[sblk == b], minlength=N)

    caps = np.full((G, NB), TIGHT, np.int64)
    caps[G - NSPILL:, :] = LOOSE

    grp = np.empty(N, np.int32)
    slot = np.empty(N, np.int32)
    for c in range(NC):
        nodes = np.arange(c * CPC, min((c + 1) * CPC, N))
        a = _pack_core(dvec[nodes], caps)
        grp[nodes] = a
        # slot within group = rank of the node among its group's members
        o = np.argsort(a, kind='stable')
        counts = np.bincount(a, minlength=G)
        starts = np.concatenate([[0], np.cumsum(counts)[:-1]])
        r = np.empty(len(nodes), np.int64)
        r[o] = np.arange(len(nodes)) - np.repeat(starts, counts)
        slot[nodes] = r
    ppos = core_of * NPC + grp.astype(np.int64) * 128 + slot

    # per-cell counts and shared chunk capacities
    g_e = grp[dst]
    c_e = core_of[dst]
    cell = (c_e * G + g_e) * NB + sblk
    cnt = np.bincount(cell, minlength=NC * G * NB).reshape(NC, G, NB)
    K_gb = np.ceil(cnt.max(axis=0) / 128).astype(np.int64)   # [G, NB]
    K_gb = np.maximum(K_gb, 1)
    KMAX = int(K_gb.max())

    sets = [list(range(s, min(s + SETG, G))) for s in range(0, G, SETG)]

    # chunk layout: calls ordered (set, block), chunks within call ordered
    # (group in set, k). chunk_base[g, b] = global chunk index of (g,b,0).
    chunk_base = np.zeros((G, NB), np.int64)
    call_info = []   # per (set, block): (slot_off, SK)
    nch = 0
    for s in sets:
        for b in range(NB):
            off = nch
            for g in s:
                chunk_base[g, b] = nch
                nch += int(K_gb[g, b])
            call_info.append((off, nch - off))
    TOTCH = nch
    TOT = TOTCH * 128

    # per-edge slot position within its core's layout
    edge_cell_base = chunk_base[g_e, sblk] * 128
    order = np.argsort(cell, kind='stable')
    counts = np.bincount(cell, minlength=NC * G * NB)
    starts = np.concatenate([[0], np.cumsum(counts)[:-1]])
    rank = np.empty(E, np.int64)
    rank[order] = np.arange(E) - np.repeat(starts, counts)
    slotpos = edge_cell_base + rank              # within core c_e's layout

    rel_src = (ppos[src] - sblk * BS).astype(np.int16)
    dst_slot = slot[dst].astype(np.float32)

    # build per-core IDX / DST arrays
    idx_arrs, dst_arrs = [], []
    for c in range(NC):
        m = c_e == c
        idxflat = np.zeros(TOT, np.int16)
        dstflat = np.full(TOT, -1.0, np.float32)
        idxflat[slotpos[m]] = rel_src[m]
        dstflat[slotpos[m]] = dst_slot[m]
        idx_w = np.tile(np.ascontiguousarray(idxflat.reshape(-1, 16).T), (8, 1))
        dst_w = np.ascontiguousarray(dstflat.reshape(TOTCH, 128).T)
        idx_arrs.append(np.ascontiguousarray(idx_w))
        dst_arrs.append(dst_w)

    # gather table: x' = dinv * x, permuted, bf16
    X1 = np.zeros((NC * NPC, DIN), np.float32)
    X1[ppos] = x * dinv[:, None]
    X1_bf = X1.astype(ml_dtypes.bfloat16)
    X1T_shards = [np.ascontiguousarray(X1_bf[c * NPC:(c + 1) * NPC].T)
                  for c in range(NC)]

    dinv_pp = np.ones((NC, 128, G), np.float32)
    dinv_pp[core_of, slot, grp] = dinv

    b1_rep = np.ascontiguousarray(np.tile(b1.astype(np.float32), (128, 1)))
    b2_rep = np.ascontiguousarray(np.tile(b2.astype(np.float32), (128, 1)))

    in_maps = []
    for c in range(NC):
        in_maps.append({
            'x1': X1_bf,
            'x1t': X1T_shards[c],
            'idx': idx_arrs[c],
            'dstloc': dst_arrs[c],
            'dinvpp': np.ascontiguousarray(dinv_pp[c]),
            'w1': np.ascontiguousarray(W1.astype(np.float32)),
            'w2': np.ascontiguousarray(W2.astype(np.float32)),
            'b1r': b1_rep,
            'b2r': b2_rep,
        })
    meta = dict(K_gb=K_gb, KMAX=KMAX, sets=sets, chunk_base=chunk_base,
                call_info=call_info, TOTCH=TOTCH, TOT=TOT)
    return in_maps, ppos, meta


def _build_nc(meta):
    K_gb = meta['K_gb']
    KMAX = meta['KMAX']
    sets = meta['sets']
    chunk_base = meta['chunk_base']
    call_info = meta['call_info']
    TOTCH = meta['TOTCH']
    TOT = meta['TOT']
    SKMAX = max(sk for _, sk in call_info)

    nc = bacc.Bacc('TRN2', target_bir_lowering=False, debug=False,
                   num_devices=NC)
    bf16, f32, i16, i32 = (mybir.dt.bfloat16, mybir.dt.float32,
                           mybir.dt.int16, mybir.dt.int32)

    x1 = nc.dram_tensor('x1', [NC * NPC, DIN], bf16, kind='ExternalInput')
    x1t = nc.dram_tensor('x1t', [DIN, NPC], bf16, kind='ExternalInput')
    idx = nc.dram_tensor('idx', [128, TOT // 16], i16, kind='ExternalInput')
    dstloc = nc.dram_tensor('dstloc', [128, TOTCH], f32, kind='ExternalInput')
    dinvpp = nc.dram_tensor('dinvpp', [128, G], f32, kind='ExternalInput')
    w1 = nc.dram_tensor('w1', [DIN, DH], f32, kind='ExternalInput')
    w2 = nc.dram_tensor('w2', [DH, DOUT], f32, kind='ExternalInput')
    b1r = nc.dram_tensor('b1r', [128, DH], f32, kind='ExternalInput')
    b2r = nc.dram_tensor('b2r', [128, DOUT], f32, kind='ExternalInput')
    out = nc.dram_tensor('out', [NPC, DOUT], f32, kind='ExternalOutput')
    import os
    dbg_enable = bool(os.environ.get('GNN_DEBUG'))
    dbg = (nc.dram_tensor('dbg', [NPC, DH], f32, kind='ExternalOutput')
           if dbg_enable else None)

    with tile.TileContext(nc) as tc:
        with ExitStack() as ctx:
            const = ctx.enter_context(tc.tile_pool(name='const', bufs=1))
            stpool = ctx.enter_context(tc.tile_pool(name='stage', bufs=2))
            mpool = ctx.enter_context(tc.tile_pool(name='mask', bufs=4))
            fpool = ctx.enter_context(tc.tile_pool(name='f32p', bufs=3))
            epool = ctx.enter_context(tc.tile_pool(name='epi', bufs=3))
            aggps = ctx.enter_context(tc.tile_pool(name='aggps', bufs=3, space='PSUM'))
            ups = ctx.enter_context(tc.tile_pool(name='ups', bufs=2, space='PSUM'))
            trps = ctx.enter_context(tc.tile_pool(name='trps', bufs=2, space='PSUM'))
            dram = ctx.enter_context(tc.tile_pool(name='dram', bufs=1, space='DRAM'))

            idx_t = const.tile([128, TOT // 16], i16)
            nc.sync.dma_start(out=idx_t[:], in_=idx[:])
            dst_t = const.tile([128, TOTCH], f32)
            nc.sync.dma_start(out=dst_t[:], in_=dstloc[:])
            dinv_t = const.tile([128, G], f32)
            nc.sync.dma_start(out=dinv_t[:], in_=dinvpp[:])
            w1_t = const.tile([DIN, DH], f32)
            nc.sync.dma_start(out=w1_t[:], in_=w1[:])
            w2_t = const.tile([DH, DOUT], f32)
            nc.sync.dma_start(out=w2_t[:], in_=w2[:])
            b1_t = const.tile([128, DH], f32)
            nc.sync.dma_start(out=b1_t[:], in_=b1r[:])
            b2_t = const.tile([128, DOUT], f32)
            nc.sync.dma_start(out=b2_t[:], in_=b2r[:])

            iota_i = const.tile([128, KMAX, 128], i32)
            nc.gpsimd.iota(iota_i[:], pattern=[[0, KMAX], [1, 128]], base=0,
                           channel_multiplier=0)
            iota_f = const.tile([128, KMAX, 128], f32)
            nc.vector.tensor_copy(out=iota_f[:], in_=iota_i[:])
            ident = const.tile([128, 128], f32)
            make_identity(nc, ident[:])

            t2t = const.tile([128, NPC], bf16)          # h1' feature-major
            t2loc = dram.tile([NPC, DIN], bf16)
            t2full = dram.tile([NC * NPC, DIN], bf16, addr_space='Shared')

            import os as _os
            _nsets = int(_os.environ.get('GNN_SETS', '1000000'))
            _noepi = bool(_os.environ.get('GNN_NOEPI'))
            _nogather = bool(_os.environ.get('GNN_NOGATHER'))
            _nomm = bool(_os.environ.get('GNN_NOMM'))

            def do_layer(layer):
                ci_call = 0
                for s in sets[:_nsets]:
                    stages = []
                    for b in range(NB):
                        off, SK = call_info[ci_call + b]
                        st = stpool.tile([128, SKMAX, 128], bf16, tag=f'st{b}')
                        src_ap = (x1 if layer == 0 else t2full[:])
                        nidx = SK * 128
                        if not _nogather:
                            nc.gpsimd.dma_gather(
                                out_ap=st[:, :SK, :],
                                in_ap=src_ap[b * BS:(b + 1) * BS, :],
                                idxs_ap=idx_t[:, off * 8: off * 8 + nidx // 16],
                                num_idxs=nidx,
                                num_idxs_reg=nidx,
                                elem_size=DIN,
                                single_packet=False,
                            )
                        else:
                            nc.gpsimd.memset(st[:, :SK, :], 0)
                        stages.append(st)
                    ci_call += NB
                    for g in s:
                        psum = aggps.tile([128, 128], f32, tag='agg')
                        nchg = int(K_gb[g].sum())
                        ci = 0
                        for b in range(NB):
                            K = int(K_gb[g, b])
                            if K == 0:
                                continue
                            cb = int(chunk_base[g, b])
                            goffb = cb - call_info[ci_call - NB + b][0]
                            mask = mpool.tile([128, KMAX, 128], bf16, tag='mask')
                            nc.vector.tensor_tensor(
                                out=mask[:, :K, :],
                                in0=dst_t[:, cb:cb + K].to_broadcast([128, K, 128]),
                                in1=iota_f[:, :K, :],
                                op=mybir.AluOpType.is_equal)
                            for k in range(K):
                                if not _nomm:
                                    nc.tensor.matmul(
                                        out=psum[:],
                                        lhsT=stages[b][:, goffb + k, :],
                                        rhs=mask[:, k, :],
                                        start=(ci == 0), stop=(ci == nchg - 1))
                                ci += 1
                        if _nomm:
                            nc.vector.memset(psum[:], 0)
                        gs = slice(g * 128, (g + 1) * 128)
                        if _noepi:
                            continue
                        aggT = fpool.tile([128, 128], f32, tag='aggT')
                        if layer == 0:
                            xT = epool.tile([128, 128], bf16, tag='xT')
                            nc.sync.dma_start(out=xT[:], in_=x1t[:, gs])
                            nc.vector.tensor_tensor(out=aggT[:], in0=psum[:],
                                                    in1=xT[:],
                                                    op=mybir.AluOpType.add)
                            up = ups.tile([128, DH], f32, tag='u')
                            nc.tensor.matmul(out=up[:], lhsT=aggT[:],
                                             rhs=w1_t[:], start=True, stop=True)
                            t1 = fpool.tile([128, DH], f32, tag='t1')
                            nc.vector.tensor_scalar(
                                out=t1[:], in0=up[:],
                                scalar1=dinv_t[:, g:g + 1], scalar2=None,
                                op0=mybir.AluOpType.mult)
                            t2 = fpool.tile([128, DH], f32, tag='t2')
                            nc.vector.tensor_tensor(out=t2[:], in0=t1[:],
                                                    in1=b1_t[:],
                                                    op=mybir.AluOpType.add)
                            # h1' = relu(t2) * dinv, in one chain
                            act = fpool.tile([128, DH], f32, tag='act')
                            nc.scalar.activation(
                                out=act[:], in_=t2[:],
                                func=mybir.ActivationFunctionType.Relu)
                            actp = fpool.tile([128, DH], f32, tag='actp')
                            nc.vector.tensor_scalar(
                                out=actp[:], in0=act[:],
                                scalar1=dinv_t[:, g:g + 1], scalar2=None,
                                op0=mybir.AluOpType.mult)
                            abf = epool.tile([128, DH], bf16, tag='abf')
                            nc.vector.tensor_copy(out=abf[:], in_=actp[:])
                            nc.sync.dma_start(out=t2loc[gs, :], in_=abf[:])
                            if dbg is not None:
                                nc.sync.dma_start(out=dbg[gs, :], in_=actp[:])
                            trp = trps.tile([128, 128], f32, tag='tr')
                            nc.tensor.transpose(out=trp[:], in_=actp[:],
                                                identity=ident[:])
                            nc.vector.tensor_copy(out=t2t[:, gs], in_=trp[:])
                        else:
                            nc.vector.tensor_tensor(out=aggT[:], in0=psum[:],
                                                    in1=t2t[:, gs],
                                                    op=mybir.AluOpType.add)
                            up_t = ups.tile([128, DH], f32, tag='u')
                            nc.tensor.matmul(out=up_t[:, :DOUT], lhsT=aggT[:],
                                             rhs=w2_t[:], start=True, stop=True)
                            t1 = fpool.tile([128, DOUT], f32, tag='o1')
                            nc.vector.tensor_scalar(
                                out=t1[:], in0=up_t[:, :DOUT],
                                scalar1=dinv_t[:, g:g + 1], scalar2=None,
                                op0=mybir.AluOpType.mult)
                            t2 = fpool.tile([128, DOUT], f32, tag='o2')
                            nc.vector.tensor_tensor(out=t2[:], in0=t1[:],
                                                    in1=b2_t[:],
                                                    op=mybir.AluOpType.add)
                            mx = epool.tile([128, 1], f32, tag='mx')
                            nc.vector.reduce_max(out=mx[:], in_=t2[:],
                                                 axis=mybir.AxisListType.X)
                            sh = fpool.tile([128, DOUT], f32, tag='sh')
                            nc.vector.tensor_scalar(
                                out=sh[:], in0=t2[:], scalar1=mx[:, 0:1],
                                scalar2=None, op0=mybir.AluOpType.subtract)
                            ex = fpool.tile([128, DOUT], f32, tag='ex')
                            se = epool.tile([128, 1], f32, tag='se')
                            nc.scalar.activation(
                                out=ex[:], in_=sh[:],
                                func=mybir.ActivationFunctionType.Exp,
                                accum_out=se[:, 0:1])
                            ls = epool.tile([128, 1], f32, tag='ls')
                            nc.scalar.activation(
                                out=ls[:], in_=se[:, 0:1],
                                func=mybir.ActivationFunctionType.Ln)
                            res = fpool.tile([128, DOUT], f32, tag='res')
                            nc.vector.tensor_scalar(
                                out=res[:], in0=sh[:], scalar1=ls[:, 0:1],
                                scalar2=None, op0=mybir.AluOpType.subtract)
                            nc.sync.dma_start(out=out[gs, :], in_=res[:])

            import os
            stage_env = os.environ.get('GNN_STAGE', 'full')
            do_layer(0)
            if stage_env in ('l1c', 'full'):
                nc.gpsimd.collective_compute(
                    'AllGather', mybir.AluOpType.bypass,
                    replica_groups=[list(range(NC))],
                    ins=[t2loc[:].opt()],
                    outs=[t2full[:].opt()])
            if stage_env == 'full':
                do_layer(1)
    nc.compile()
    return nc


def kernel(x, edge_index, W1, b1, W2, b2):
    x = np.asarray(x, dtype=np.float32)
    edge_index = np.asarray(edge_index)
    in_maps, ppos, meta = _preprocess(x, edge_index, np.asarray(W1),
                                      np.asarray(b1), np.asarray(W2),
                                      np.asarray(b2))
    key = (meta['TOT'], meta['KMAX'], meta['K_gb'].tobytes())
    if key not in _cache:
        _cache[key] = _build_nc(meta)
    nc = _cache[key]
    import os
    trace = bool(os.environ.get('GNN_TRACE'))
    r = run_bass_kernel_spmd(nc, in_maps, core_ids=list(range(NC)),
                             trace=trace)
    kernel.last_exec_ns = r.exec_time_ns
    kernel.last_results = r
    full = np.concatenate([r.results[c]['out'] for c in range(NC)], axis=0)
    return np.ascontiguousarray(full[ppos]).astype(np.float32)



# revision 5
# speedup vs baseline: 1.6674x; 1.6674x over previous
"""2-layer GCN (PyG GCNConv x2 + log_softmax) on 8 Trainium2 NeuronCores.

Strategy:
  - Nodes are permuted and sharded across 8 cores (12544 slots/core, 98 groups
    of 128). A per-core vector bin-packing balances, for every (group, source
    block) cell, the number of in-edges, so the static chunk layout (shared by
    all SPMD cores) wastes only ~2% padding.
  - Aggregation A_norm @ Z is commuted past the dense transforms:
        out_i = dinv_i * ((sum_{j->i} z'_j + z'_i) @ W) + b,   z' = dinv .* z
    so each layer is: gather z' rows by edge source (dma_gather, bf16 table),
    segment-sum per 128-node destination group via a 0/1 mask matmul
    accumulated in PSUM (feature-major), add the self row, dense-transform by
    W (fp32), scale+bias+activation, log_softmax at the end.
  - Gather tables are indexed with int16 relative indices inside 4 source
    blocks of 25088 rows (= 2 cores' shards). Both layers reuse the same
    edge layout/indices; layer-2's table (h1') is built on-device and
    exchanged with a single AllGather collective.
"""
import sys
sys.path.insert(0, '/opt/trn_rl_repo')

import numpy as np
import ml_dtypes
from contextlib import ExitStack

from concourse import bass, bacc, mybir, tile
from concourse.bass_utils import run_bass_kernel_spmd
from concourse.masks import make_identity

N = 100000
E = 1600000
DIN, DH, DOUT = 128, 128, 64
NC = 8
CPC = 12500            # real nodes per core
NPC = 12544            # padded slots per core (98 * 128)
G = 98                 # node groups of 128 per core
BS = 25088             # source block size (NPC * NC / 4), int16-addressable
NB = 4                 # source blocks
TIGHT, LOOSE, NSPILL = 512, 640, 6
SETG = 10              # groups per gather-call set

_cache = {}


def _pack_core(dv, caps):
    """Greedy vector bin-packing: nodes (rows of dv, [n,NB]) into G bins of
    128 slots with per-block capacity caps [G,NB]. Returns bin assignment."""
    n = dv.shape[0]
    order = np.argsort(-dv.sum(1), kind='stable')
    rem = caps.astype(np.int64).copy()
    slots = np.full(G, 128, np.int64)
    assign = np.empty(n, np.int32)
    for i in order:
        d = dv[i]
        ok = (slots > 0) & (rem >= d[None, :]).all(1)
        if ok.any():
            cand = np.where(ok)[0]
            sl = (rem[cand] - d[None, :]).min(1)
            j = cand[np.argmax(sl * 1000 + slots[cand])]
        else:
            cand = np.where(slots > 0)[0]
            over = np.maximum(d[None, :] - rem[cand], 0).sum(1)
            j = cand[np.argmin(over)]
        assign[i] = j
        rem[j] -= d
        slots[j] -= 1
    return assign


def _preprocess(x, edge_index, W1, b1, W2, b2):
    src = edge_index[0].astype(np.int64)
    dst = edge_index[1].astype(np.int64)
    indeg = np.bincount(dst, minlength=N)
    dinv = (1.0 / np.sqrt((indeg + 1).astype(np.float64))).astype(np.float32)

    core_of = np.minimum(np.arange(N) // CPC, NC - 1)
    blk_of = core_of // 2                       # source block = core pair
    sblk = blk_of[src]

    # demand vector: per node, in-edges split by source block
    dvec = np.zeros((N, NB), np.int32)
    for b in range(NB):
        dvec[:, b] = np.bincount(dst[sblk == b], minlength=N)

    caps = np.full((G, NB), TIGHT, np.int64)
    caps[G - NSPILL:, :] = LOOSE

    grp = np.empty(N, np.int32)
    slot = np.empty(N, np.int32)
    for c in range(NC):
        nodes = np.arange(c * CPC, min((c + 1) * CPC, N))
        a = _pack_core(dvec[nodes], caps)
        grp[nodes] = a
        # slot within group = rank of the node among its group's members
        o = np.argsort(a, kind='stable')
        counts = np.bincount(a, minlength=G)
        starts = np.concatenate([[0], np.cumsum(counts)[:-1]])
        r = np.empty(len(nodes), np.int64)
        r[o] = np.arange(len(nodes)) - np.repeat(starts, counts)
        slot[nodes] = r
    ppos = core_of * NPC + grp.astype(np.int64) * 128 + slot

    # per-cell counts and shared chunk capacities
    g_e = grp[dst]
    c_e = core_of[dst]
    cell = (c_e * G + g_e) * NB + sblk
    cnt = np.bincount(cell, minlength=NC * G * NB).reshape(NC, G, NB)
    K_gb = np.ceil(cnt.max(axis=0) / 128).astype(np.int64)   # [G, NB]
    K_gb = np.maximum(K_gb, 1)
    KMAX = int(K_gb.max())

    sets = [list(range(s, min(s + SETG, G))) for s in range(0, G, SETG)]

    # chunk layout: calls ordered (set, block), chunks within call ordered
    # (group in set, k). chunk_base[g, b] = global chunk index of (g,b,0).
    chunk_base = np.zeros((G, NB), np.int64)
    call_info = []   # per (set, block): (slot_off, SK)
    nch = 0
    for s in sets:
        for b in range(NB):
            off = nch
            for g in s:
                chunk_base[g, b] = nch
                nch += int(K_gb[g, b])
            call_info.append((off, nch - off))
    TOTCH = nch
    TOT = TOTCH * 128

    # per-edge slot position within its core's layout
    edge_cell_base = chunk_base[g_e, sblk] * 128
    order = np.argsort(cell, kind='stable')
    counts = np.bincount(cell, minlength=NC * G * NB)
    starts = np.concatenate([[0], np.cumsum(counts)[:-1]])
    rank = np.empty(E, np.int64)
    rank[order] = np.arange(E) - np.repeat(starts, counts)
    slotpos = edge_cell_base + rank              # within core c_e's layout

    rel_src = (ppos[src] - sblk * BS).astype(np.int16)
    dst_slot = slot[dst].astype(np.float32)

    # build per-core IDX / DST arrays
    idx_arrs, dst_arrs = [], []
    for c in range(NC):
        m = c_e == c
        idxflat = np.zeros(TOT, np.int16)
        dstflat = np.full(TOT, -1.0, np.float32)
        idxflat[slotpos[m]] = rel_src[m]
        dstflat[slotpos[m]] = dst_slot[m]
        idx_w = np.tile(np.ascontiguousarray(idxflat.reshape(-1, 16).T), (8, 1))
        dst_w = np.ascontiguousarray(dstflat.reshape(TOTCH, 128).T)
        idx_arrs.append(np.ascontiguousarray(idx_w))
        dst_arrs.append(dst_w)

    # gather table: x' = dinv * x, permuted, bf16
    X1 = np.zeros((NC * NPC, DIN), np.float32)
    X1[ppos] = x * dinv[:, None]
    X1_bf = X1.astype(ml_dtypes.bfloat16)
    X1T_shards = [np.ascontiguousarray(X1_bf[c * NPC:(c + 1) * NPC].T)
                  for c in range(NC)]

    dinv_pp = np.ones((NC, 128, G), np.float32)
    dinv_pp[core_of, slot, grp] = dinv

    b1_rep = np.ascontiguousarray(np.tile(b1.astype(np.float32), (128, 1)))
    b2_rep = np.ascontiguousarray(np.tile(b2.astype(np.float32), (128, 1)))

    in_maps = []
    for c in range(NC):
        in_maps.append({
            'x1': X1_bf,
            'x1t': X1T_shards[c],
            'idx': idx_arrs[c],
            'dstloc': dst_arrs[c],
            'dinvpp': np.ascontiguousarray(dinv_pp[c]),
            'w1': np.ascontiguousarray(W1.astype(np.float32)),
            'w2': np.ascontiguousarray(W2.astype(np.float32)),
            'b1r': b1_rep,
            'b2r': b2_rep,
        })
    meta = dict(K_gb=K_gb, KMAX=KMAX, sets=sets, chunk_base=chunk_base,
                call_info=call_info, TOTCH=TOTCH, TOT=TOT)
    return in_maps, ppos, meta


def _build_nc(meta):
    K_gb = meta['K_gb']
    KMAX = meta['KMAX']
    sets = meta['sets']
    chunk_base = meta['chunk_base']
    call_info = meta['call_info']
    TOTCH = meta['TOTCH']
    TOT = meta['TOT']
    SKMAX = max(sk for _, sk in call_info)

    import os as _os0
    _nq = int(_os0.environ.get('GNN_NQ', '4'))
    nc = bacc.Bacc('TRN2', target_bir_lowering=False, debug=False,
                   num_devices=NC, num_swdge_queues=_nq)
    bf16, f32, i16, i32 = (mybir.dt.bfloat16, mybir.dt.float32,
                           mybir.dt.int16, mybir.dt.int32)

    x1 = nc.dram_tensor('x1', [NC * NPC, DIN], bf16, kind='ExternalInput')
    x1t = nc.dram_tensor('x1t', [DIN, NPC], bf16, kind='ExternalInput')
    idx = nc.dram_tensor('idx', [128, TOT // 16], i16, kind='ExternalInput')
    dstloc = nc.dram_tensor('dstloc', [128, TOTCH], f32, kind='ExternalInput')
    dinvpp = nc.dram_tensor('dinvpp', [128, G], f32, kind='ExternalInput')
    w1 = nc.dram_tensor('w1', [DIN, DH], f32, kind='ExternalInput')
    w2 = nc.dram_tensor('w2', [DH, DOUT], f32, kind='ExternalInput')
    b1r = nc.dram_tensor('b1r', [128, DH], f32, kind='ExternalInput')
    b2r = nc.dram_tensor('b2r', [128, DOUT], f32, kind='ExternalInput')
    out = nc.dram_tensor('out', [NPC, DOUT], f32, kind='ExternalOutput')
    import os
    dbg_enable = bool(os.environ.get('GNN_DEBUG'))
    dbg = (nc.dram_tensor('dbg', [NPC, DH], f32, kind='ExternalOutput')
           if dbg_enable else None)

    with tile.TileContext(nc) as tc:
        with ExitStack() as ctx:
            const = ctx.enter_context(tc.tile_pool(name='const', bufs=1))
            stpool = ctx.enter_context(tc.tile_pool(name='stage', bufs=2))
            mpool = ctx.enter_context(tc.tile_pool(name='mask', bufs=4))
            fpool = ctx.enter_context(tc.tile_pool(name='f32p', bufs=3))
            epool = ctx.enter_context(tc.tile_pool(name='epi', bufs=3))
            aggps = ctx.enter_context(tc.tile_pool(name='aggps', bufs=3, space='PSUM'))
            ups = ctx.enter_context(tc.tile_pool(name='ups', bufs=2, space='PSUM'))
            trps = ctx.enter_context(tc.tile_pool(name='trps', bufs=2, space='PSUM'))
            dram = ctx.enter_context(tc.tile_pool(name='dram', bufs=1, space='DRAM'))

            idx_t = const.tile([128, TOT // 16], i16)
            nc.sync.dma_start(out=idx_t[:], in_=idx[:])
            dst_t = const.tile([128, TOTCH], f32)
            nc.sync.dma_start(out=dst_t[:], in_=dstloc[:])
            dinv_t = const.tile([128, G], f32)
            nc.sync.dma_start(out=dinv_t[:], in_=dinvpp[:])
            w1_t = const.tile([DIN, DH], f32)
            nc.sync.dma_start(out=w1_t[:], in_=w1[:])
            w2_t = const.tile([DH, DOUT], f32)
            nc.sync.dma_start(out=w2_t[:], in_=w2[:])
            b1_t = const.tile([128, DH], f32)
            nc.sync.dma_start(out=b1_t[:], in_=b1r[:])
            b2_t = const.tile([128, DOUT], f32)
            nc.sync.dma_start(out=b2_t[:], in_=b2r[:])

            iota_i = const.tile([128, KMAX, 128], i32)
            nc.gpsimd.iota(iota_i[:], pattern=[[0, KMAX], [1, 128]], base=0,
                           channel_multiplier=0)
            iota_f = const.tile([128, KMAX, 128], f32)
            nc.vector.tensor_copy(out=iota_f[:], in_=iota_i[:])
            ident = const.tile([128, 128], f32)
            make_identity(nc, ident[:])

            sh_all = const.tile([128, G, DOUT], f32)    # layer-2 shifted logits
            se_all = const.tile([128, G], f32)          # layer-2 exp sums
            t2t = const.tile([128, NPC], bf16)          # h1' feature-major
            t2loc = dram.tile([NPC, DIN], bf16)
            t2full = dram.tile([NC * NPC, DIN], bf16, addr_space='Shared')

            import os as _os
            _nsets = int(_os.environ.get('GNN_SETS', '1000000'))
            _noepi = bool(_os.environ.get('GNN_NOEPI'))
            _nogather = bool(_os.environ.get('GNN_NOGATHER'))
            _nomm = bool(_os.environ.get('GNN_NOMM'))

            def do_layer(layer):
                ci_call = 0
                for s in sets[:_nsets]:
                    stages = []
                    for b in range(NB):
                        off, SK = call_info[ci_call + b]
                        st = stpool.tile([128, SKMAX, 128], bf16, tag=f'st{b}')
                        src_ap = (x1 if layer == 0 else t2full[:])
                        nidx = SK * 128
                        if not _nogather:
                            nc.gpsimd.dma_gather(
                                out_ap=st[:, :SK, :],
                                in_ap=src_ap[b * BS:(b + 1) * BS, :],
                                idxs_ap=idx_t[:, off * 8: off * 8 + nidx // 16],
                                num_idxs=nidx,
                                num_idxs_reg=nidx,
                                elem_size=DIN,
                                single_packet=False,
                                queue_num=b % _nq,
                            )
                        else:
                            nc.gpsimd.memset(st[:, :SK, :], 0)
                        stages.append(st)
                    ci_call += NB
                    for g in s:
                        psum = aggps.tile([128, 128], f32, tag='agg')
                        nchg = int(K_gb[g].sum())
                        ci = 0
                        for b in range(NB):
                            K = int(K_gb[g, b])
                            if K == 0:
                                continue
                            cb = int(chunk_base[g, b])
                            goffb = cb - call_info[ci_call - NB + b][0]
                            mask = mpool.tile([128, KMAX, 128], bf16, tag='mask')
                            nc.vector.tensor_tensor(
                                out=mask[:, :K, :],
                                in0=dst_t[:, cb:cb + K].to_broadcast([128, K, 128]),
                                in1=iota_f[:, :K, :],
                                op=mybir.AluOpType.is_equal)
                            for k in range(K):
                                if not _nomm:
                                    nc.tensor.matmul(
                                        out=psum[:],
                                        lhsT=stages[b][:, goffb + k, :],
                                        rhs=mask[:, k, :],
                                        start=(ci == 0), stop=(ci == nchg - 1))
                                ci += 1
                        if _nomm:
                            nc.vector.memset(psum[:], 0)
                        gs = slice(g * 128, (g + 1) * 128)
                        if _noepi:
                            continue
                        aggT = fpool.tile([128, 128], f32, tag='aggT')
                        if layer == 0:
                            xT = epool.tile([128, 128], bf16, tag='xT')
                            nc.sync.dma_start(out=xT[:], in_=x1t[:, gs])
                            nc.vector.tensor_tensor(out=aggT[:], in0=psum[:],
                                                    in1=xT[:],
                                                    op=mybir.AluOpType.add)
                            up = ups.tile([128, DH], f32, tag='u')
                            nc.tensor.matmul(out=up[:], lhsT=aggT[:],
                                             rhs=w1_t[:], start=True, stop=True)
                            t1 = fpool.tile([128, DH], f32, tag='t1')
                            nc.vector.tensor_scalar(
                                out=t1[:], in0=up[:],
                                scalar1=dinv_t[:, g:g + 1], scalar2=None,
                                op0=mybir.AluOpType.mult)
                            t2 = fpool.tile([128, DH], f32, tag='t2')
                            nc.vector.tensor_tensor(out=t2[:], in0=t1[:],
                                                    in1=b1_t[:],
                                                    op=mybir.AluOpType.add)
                            # h1' = relu(t2) * dinv, in one chain
                            act = fpool.tile([128, DH], f32, tag='act')
                            nc.scalar.activation(
                                out=act[:], in_=t2[:],
                                func=mybir.ActivationFunctionType.Relu)
                            actp = fpool.tile([128, DH], f32, tag='actp')
                            nc.vector.tensor_scalar(
                                out=actp[:], in0=act[:],
                                scalar1=dinv_t[:, g:g + 1], scalar2=None,
                                op0=mybir.AluOpType.mult)
                            abf = epool.tile([128, DH], bf16, tag='abf')
                            nc.vector.tensor_copy(out=abf[:], in_=actp[:])
                            nc.sync.dma_start(out=t2loc[gs, :], in_=abf[:])
                            if dbg is not None:
                                nc.sync.dma_start(out=dbg[gs, :], in_=actp[:])
                            trp = trps.tile([128, 128], f32, tag='tr')
                            nc.tensor.transpose(out=trp[:], in_=actp[:],
                                                identity=ident[:])
                            nc.vector.tensor_copy(out=t2t[:, gs], in_=trp[:])
                        else:
                            nc.vector.tensor_tensor(out=aggT[:], in0=psum[:],
                                                    in1=t2t[:, gs],
                                                    op=mybir.AluOpType.add)
                            up_t = ups.tile([128, DH], f32, tag='u')
                            nc.tensor.matmul(out=up_t[:, :DOUT], lhsT=aggT[:],
                                             rhs=w2_t[:], start=True, stop=True)
                            t1 = fpool.tile([128, DOUT], f32, tag='o1')
                            nc.vector.tensor_scalar(
                                out=t1[:], in0=up_t[:, :DOUT],
                                scalar1=dinv_t[:, g:g + 1], scalar2=None,
                                op0=mybir.AluOpType.mult)
                            t2 = fpool.tile([128, DOUT], f32, tag='o2')
                            nc.vector.tensor_tensor(out=t2[:], in0=t1[:],
                                                    in1=b2_t[:],
                                                    op=mybir.AluOpType.add)
                            mx = epool.tile([128, 1], f32, tag='mx')
                            nc.vector.reduce_max(out=mx[:], in_=t2[:],
                                                 axis=mybir.AxisListType.X)
                            nc.vector.tensor_scalar(
                                out=sh_all[:, g, :], in0=t2[:],
                                scalar1=mx[:, 0:1],
                                scalar2=None, op0=mybir.AluOpType.subtract)
                            ex = fpool.tile([128, DOUT], f32, tag='ex')
                            nc.scalar.activation(
                                out=ex[:], in_=sh_all[:, g, :],
                                func=mybir.ActivationFunctionType.Exp,
                                accum_out=se_all[:, g:g + 1])

            import os
            stage_env = os.environ.get('GNN_STAGE', 'full')
            do_layer(0)
            if stage_env in ('l1c', 'full'):
                nc.gpsimd.collective_compute(
                    'AllGather', mybir.AluOpType.bypass,
                    replica_groups=[list(range(NC))],
                    ins=[t2loc[:].opt()],
                    outs=[t2full[:].opt()])
            if stage_env == 'full':
                do_layer(1)
    nc.compile()
    return nc


def kernel(x, edge_index, W1, b1, W2, b2):
    x = np.asarray(x, dtype=np.float32)
    edge_index = np.asarray(edge_index)
    in_maps, ppos, meta = _preprocess(x, edge_index, np.asarray(W1),
                                      np.asarray(b1), np.asarray(W2),
                                      np.asarray(b2))
    key = (meta['TOT'], meta['KMAX'], meta['K_gb'].tobytes())
    if key not in _cache:
        _cache[key] = _build_nc(meta)
    nc = _cache[key]
    import os
    trace = bool(os.environ.get('GNN_TRACE'))
    r = run_bass_kernel_spmd(nc, in_maps, core_ids=list(range(NC)),
                             trace=trace)
    kernel.last_exec_ns = r.exec_time_ns
    kernel.last_results = r
    full = np.concatenate([r.results[c]['out'] for c in range(NC)], axis=0)
    return np.ascontiguousarray(full[ppos]).astype(np.float32)



# revision 6
# speedup vs baseline: 2.0681x; 1.2403x over previous
"""2-layer GCN (PyG GCNConv x2 + log_softmax) on 8 Trainium2 NeuronCores.

Strategy:
  - Nodes are permuted and sharded across 8 cores (12544 slots/core, 98 groups
    of 128). A per-core vector bin-packing balances, for every (group, source
    block) cell, the number of in-edges, so the static chunk layout (shared by
    all SPMD cores) wastes only ~2% padding.
  - Aggregation A_norm @ Z is commuted past the dense transforms:
        out_i = dinv_i * ((sum_{j->i} z'_j + z'_i) @ W) + b,   z' = dinv .* z
    so each layer is: gather z' rows by edge source (dma_gather, bf16 table),
    segment-sum per 128-node destination group via a 0/1 mask matmul
    accumulated in PSUM (feature-major), add the self row, dense-transform by
    W (fp32), scale+bias+activation, log_softmax at the end.
  - Gather tables are indexed with int16 relative indices inside 4 source
    blocks of 25088 rows (= 2 cores' shards). Both layers reuse the same
    edge layout/indices; layer-2's table (h1') is built on-device and
    exchanged with a single AllGather collective.
"""
import sys
sys.path.insert(0, '/opt/trn_rl_repo')

import numpy as np
import ml_dtypes
from contextlib import ExitStack

from concourse import bass, bacc, mybir, tile
from concourse.bass_utils import run_bass_kernel_spmd
from concourse.masks import make_identity

N = 100000
E = 1600000
DIN, DH, DOUT = 128, 128, 64
NC = 8
CPC = 12500            # real nodes per core
NPC = 12544            # padded slots per core (98 * 128)
G = 98                 # node groups of 128 per core
BS = 25088             # source block size (NPC * NC / 4), int16-addressable
NB = 4                 # source blocks
TIGHT, LOOSE, NSPILL = 512, 640, 6
SETG = 10              # groups per gather-call set

_cache = {}


def _pack_core(dv, caps):
    """Greedy vector bin-packing: nodes (rows of dv, [n,NB]) into G bins of
    128 slots with per-block capacity caps [G,NB]. Returns bin assignment."""
    n = dv.shape[0]
    order = np.argsort(-dv.sum(1), kind='stable')
    rem = caps.astype(np.int64).copy()
    slots = np.full(G, 128, np.int64)
    assign = np.empty(n, np.int32)
    for i in order:
        d = dv[i]
        ok = (slots > 0) & (rem >= d[None, :]).all(1)
        if ok.any():
            cand = np.where(ok)[0]
            sl = (rem[cand] - d[None, :]).min(1)
            j = cand[np.argmax(sl * 1000 + slots[cand])]
        else:
            cand = np.where(slots > 0)[0]
            over = np.maximum(d[None, :] - rem[cand], 0).sum(1)
            j = cand[np.argmin(over)]
        assign[i] = j
        rem[j] -= d
        slots[j] -= 1
    return assign


def _preprocess(x, edge_index, W1, b1, W2, b2):
    src = edge_index[0].astype(np.int64)
    dst = edge_index[1].astype(np.int64)
    indeg = np.bincount(dst, minlength=N)
    dinv = (1.0 / np.sqrt((indeg + 1).astype(np.float64))).astype(np.float32)

    core_of = np.minimum(np.arange(N) // CPC, NC - 1)
    blk_of = core_of // 2                       # source block = core pair
    sblk = blk_of[src]

    # demand vector: per node, in-edges split by source block
    dvec = np.zeros((N, NB), np.int32)
    for b in range(NB):
        dvec[:, b] = np.bincount(dst[sblk == b], minlength=N)

    caps = np.full((G, NB), TIGHT, np.int64)
    caps[G - NSPILL:, :] = LOOSE

    grp = np.empty(N, np.int32)
    slot = np.empty(N, np.int32)
    for c in range(NC):
        nodes = np.arange(c * CPC, min((c + 1) * CPC, N))
        a = _pack_core(dvec[nodes], caps)
        grp[nodes] = a
        # slot within group = rank of the node among its group's members
        o = np.argsort(a, kind='stable')
        counts = np.bincount(a, minlength=G)
        starts = np.concatenate([[0], np.cumsum(counts)[:-1]])
        r = np.empty(len(nodes), np.int64)
        r[o] = np.arange(len(nodes)) - np.repeat(starts, counts)
        slot[nodes] = r
    ppos = core_of * NPC + grp.astype(np.int64) * 128 + slot

    # per-cell counts and shared chunk capacities
    g_e = grp[dst]
    c_e = core_of[dst]
    cell = (c_e * G + g_e) * NB + sblk
    cnt = np.bincount(cell, minlength=NC * G * NB).reshape(NC, G, NB)
    K_gb = np.ceil(cnt.max(axis=0) / 128).astype(np.int64)   # [G, NB]
    K_gb = np.maximum(K_gb, 1)
    KMAX = int(K_gb.max())

    sets = [list(range(s, min(s + SETG, G))) for s in range(0, G, SETG)]

    # chunk layout: calls ordered (set, block), chunks within call ordered
    # (group in set, k). chunk_base[g, b] = global chunk index of (g,b,0).
    chunk_base = np.zeros((G, NB), np.int64)
    call_info = []   # per (set, block): (slot_off, SK)
    nch = 0
    for s in sets:
        for b in range(NB):
            off = nch
            for g in s:
                chunk_base[g, b] = nch
                nch += int(K_gb[g, b])
            call_info.append((off, nch - off))
    TOTCH = nch
    TOT = TOTCH * 128

    # per-edge slot position within its core's layout
    edge_cell_base = chunk_base[g_e, sblk] * 128
    order = np.argsort(cell, kind='stable')
    counts = np.bincount(cell, minlength=NC * G * NB)
    starts = np.concatenate([[0], np.cumsum(counts)[:-1]])
    rank = np.empty(E, np.int64)
    rank[order] = np.arange(E) - np.repeat(starts, counts)
    slotpos = edge_cell_base + rank              # within core c_e's layout

    rel_src = (ppos[src] - sblk * BS).astype(np.int16)
    dst_slot = slot[dst].astype(np.float32)

    # build per-core IDX / DST arrays
    idx_arrs, dst_arrs = [], []
    for c in range(NC):
        m = c_e == c
        idxflat = np.zeros(TOT, np.int16)
        dstflat = np.full(TOT, -1.0, np.float32)
        idxflat[slotpos[m]] = rel_src[m]
        dstflat[slotpos[m]] = dst_slot[m]
        idx_w = np.tile(np.ascontiguousarray(idxflat.reshape(-1, 16).T), (8, 1))
        dst_w = np.ascontiguousarray(dstflat.reshape(TOTCH, 128).T)
        idx_arrs.append(np.ascontiguousarray(idx_w))
        dst_arrs.append(dst_w)

    # gather table: x' = dinv * x, permuted, bf16
    X1 = np.zeros((NC * NPC, DIN), np.float32)
    X1[ppos] = x * dinv[:, None]
    X1_bf = X1.astype(ml_dtypes.bfloat16)
    X1T_shards = [np.ascontiguousarray(X1_bf[c * NPC:(c + 1) * NPC].T)
                  for c in range(NC)]

    dinv_pp = np.ones((NC, 128, G), np.float32)
    dinv_pp[core_of, slot, grp] = dinv

    b1_rep = np.ascontiguousarray(np.tile(b1.astype(np.float32), (128, 1)))
    b2_rep = np.ascontiguousarray(np.tile(b2.astype(np.float32), (128, 1)))

    in_maps = []
    for c in range(NC):
        in_maps.append({
            'x1': X1_bf,
            'x1t': X1T_shards[c],
            'idx': idx_arrs[c],
            'dstloc': dst_arrs[c],
            'dinvpp': np.ascontiguousarray(dinv_pp[c]),
            'w1': np.ascontiguousarray(W1.astype(np.float32)),
            'w2': np.ascontiguousarray(W2.astype(np.float32)),
            'b1r': b1_rep,
            'b2r': b2_rep,
        })
    meta = dict(K_gb=K_gb, KMAX=KMAX, sets=sets, chunk_base=chunk_base,
                call_info=call_info, TOTCH=TOTCH, TOT=TOT)
    return in_maps, ppos, meta


def _build_nc(meta):
    K_gb = meta['K_gb']
    KMAX = meta['KMAX']
    sets = meta['sets']
    chunk_base = meta['chunk_base']
    call_info = meta['call_info']
    TOTCH = meta['TOTCH']
    TOT = meta['TOT']
    SKMAX = max(sk for _, sk in call_info)

    import os as _os0
    _nq = int(_os0.environ.get('GNN_NQ', '4'))
    nc = bacc.Bacc('TRN2', target_bir_lowering=False, debug=False,
                   num_devices=NC, num_swdge_queues=_nq)
    bf16, f32, i16, i32 = (mybir.dt.bfloat16, mybir.dt.float32,
                           mybir.dt.int16, mybir.dt.int32)

    x1 = nc.dram_tensor('x1', [NC * NPC, DIN], bf16, kind='ExternalInput')
    x1t = nc.dram_tensor('x1t', [DIN, NPC], bf16, kind='ExternalInput')
    idx = nc.dram_tensor('idx', [128, TOT // 16], i16, kind='ExternalInput')
    dstloc = nc.dram_tensor('dstloc', [128, TOTCH], f32, kind='ExternalInput')
    dinvpp = nc.dram_tensor('dinvpp', [128, G], f32, kind='ExternalInput')
    w1 = nc.dram_tensor('w1', [DIN, DH], f32, kind='ExternalInput')
    w2 = nc.dram_tensor('w2', [DH, DOUT], f32, kind='ExternalInput')
    b1r = nc.dram_tensor('b1r', [128, DH], f32, kind='ExternalInput')
    b2r = nc.dram_tensor('b2r', [128, DOUT], f32, kind='ExternalInput')
    out = nc.dram_tensor('out', [NPC, DOUT], f32, kind='ExternalOutput')
    import os
    dbg_enable = bool(os.environ.get('GNN_DEBUG'))
    dbg = (nc.dram_tensor('dbg', [NPC, DH], f32, kind='ExternalOutput')
           if dbg_enable else None)

    with tile.TileContext(nc) as tc:
        with ExitStack() as ctx:
            const = ctx.enter_context(tc.tile_pool(name='const', bufs=1))
            stpool = ctx.enter_context(tc.tile_pool(name='stage', bufs=2))
            mpool = ctx.enter_context(tc.tile_pool(name='mask', bufs=4))
            fpool = ctx.enter_context(tc.tile_pool(name='f32p', bufs=3))
            epool = ctx.enter_context(tc.tile_pool(name='epi', bufs=3))
            aggps = ctx.enter_context(tc.tile_pool(name='aggps', bufs=3, space='PSUM'))
            ups = ctx.enter_context(tc.tile_pool(name='ups', bufs=2, space='PSUM'))
            trps = ctx.enter_context(tc.tile_pool(name='trps', bufs=2, space='PSUM'))
            dram = ctx.enter_context(tc.tile_pool(name='dram', bufs=1, space='DRAM'))

            idx_t = const.tile([128, TOT // 16], i16)
            nc.sync.dma_start(out=idx_t[:], in_=idx[:])
            dst_t = const.tile([128, TOTCH], f32)
            nc.sync.dma_start(out=dst_t[:], in_=dstloc[:])
            dinv_t = const.tile([128, G], f32)
            nc.sync.dma_start(out=dinv_t[:], in_=dinvpp[:])
            w1_t = const.tile([DIN, DH], f32)
            nc.sync.dma_start(out=w1_t[:], in_=w1[:])
            w2_t = const.tile([DH, DOUT], f32)
            nc.sync.dma_start(out=w2_t[:], in_=w2[:])
            b1_t = const.tile([128, DH], f32)
            nc.sync.dma_start(out=b1_t[:], in_=b1r[:])
            b2_t = const.tile([128, DOUT], f32)
            nc.sync.dma_start(out=b2_t[:], in_=b2r[:])

            iota_i = const.tile([128, KMAX, 128], i32)
            nc.gpsimd.iota(iota_i[:], pattern=[[0, KMAX], [1, 128]], base=0,
                           channel_multiplier=0)
            iota_f = const.tile([128, KMAX, 128], f32)
            nc.vector.tensor_copy(out=iota_f[:], in_=iota_i[:])
            ident = const.tile([128, 128], f32)
            make_identity(nc, ident[:])

            sh_all = const.tile([128, G, DOUT], f32)    # layer-2 shifted logits
            se_all = const.tile([128, G], f32)          # layer-2 exp sums
            t2t = const.tile([128, NPC], bf16)          # h1' feature-major
            t2loc = dram.tile([NPC, DIN], bf16)
            t2full = dram.tile([NC * NPC, DIN], bf16, addr_space='Shared')

            import os as _os
            _nsets = int(_os.environ.get('GNN_SETS', '1000000'))
            _noepi = bool(_os.environ.get('GNN_NOEPI'))
            _nogather = bool(_os.environ.get('GNN_NOGATHER'))
            _nomm = bool(_os.environ.get('GNN_NOMM'))

            def do_layer(layer):
                ci_call = 0
                for s in sets[:_nsets]:
                    stages = []
                    for b in range(NB):
                        off, SK = call_info[ci_call + b]
                        st = stpool.tile([128, SKMAX, 128], bf16, tag=f'st{b}')
                        src_ap = (x1 if layer == 0 else t2full[:])
                        nidx = SK * 128
                        if not _nogather:
                            nc.gpsimd.dma_gather(
                                out_ap=st[:, :SK, :],
                                in_ap=src_ap[b * BS:(b + 1) * BS, :],
                                idxs_ap=idx_t[:, off * 8: off * 8 + nidx // 16],
                                num_idxs=nidx,
                                num_idxs_reg=nidx,
                                elem_size=DIN,
                                single_packet=False,
                                queue_num=b % _nq,
                            )
                        else:
                            nc.gpsimd.memset(st[:, :SK, :], 0)
                        stages.append(st)
                    ci_call += NB
                    for g in s:
                        psum = aggps.tile([128, 128], f32, tag='agg')
                        nchg = int(K_gb[g].sum())
                        ci = 0
                        for b in range(NB):
                            K = int(K_gb[g, b])
                            if K == 0:
                                continue
                            cb = int(chunk_base[g, b])
                            goffb = cb - call_info[ci_call - NB + b][0]
                            mask = mpool.tile([128, KMAX, 128], bf16, tag='mask')
                            nc.vector.tensor_tensor(
                                out=mask[:, :K, :],
                                in0=dst_t[:, cb:cb + K].to_broadcast([128, K, 128]),
                                in1=iota_f[:, :K, :],
                                op=mybir.AluOpType.is_equal)
                            for k in range(K):
                                if not _nomm:
                                    nc.tensor.matmul(
                                        out=psum[:],
                                        lhsT=stages[b][:, goffb + k, :],
                                        rhs=mask[:, k, :],
                                        start=(ci == 0), stop=(ci == nchg - 1))
                                ci += 1
                        if _nomm:
                            nc.vector.memset(psum[:], 0)
                        gs = slice(g * 128, (g + 1) * 128)
                        if _noepi:
                            continue
                        aggT = fpool.tile([128, 128], f32, tag='aggT')
                        if layer == 0:
                            xT = epool.tile([128, 128], bf16, tag='xT')
                            nc.sync.dma_start(out=xT[:], in_=x1t[:, gs])
                            nc.vector.tensor_tensor(out=aggT[:], in0=psum[:],
                                                    in1=xT[:],
                                                    op=mybir.AluOpType.add)
                            up = ups.tile([128, DH], f32, tag='u')
                            nc.tensor.matmul(out=up[:], lhsT=aggT[:],
                                             rhs=w1_t[:], start=True, stop=True)
                            t1 = fpool.tile([128, DH], f32, tag='t1')
                            nc.vector.tensor_scalar(
                                out=t1[:], in0=up[:],
                                scalar1=dinv_t[:, g:g + 1], scalar2=None,
                                op0=mybir.AluOpType.mult)
                            t2 = fpool.tile([128, DH], f32, tag='t2')
                            nc.vector.tensor_tensor(out=t2[:], in0=t1[:],
                                                    in1=b1_t[:],
                                                    op=mybir.AluOpType.add)
                            # h1' = relu(t2) * dinv, in one chain
                            act = fpool.tile([128, DH], f32, tag='act')
                            nc.scalar.activation(
                                out=act[:], in_=t2[:],
                                func=mybir.ActivationFunctionType.Relu)
                            actp = fpool.tile([128, DH], f32, tag='actp')
                            nc.vector.tensor_scalar(
                                out=actp[:], in0=act[:],
                                scalar1=dinv_t[:, g:g + 1], scalar2=None,
                                op0=mybir.AluOpType.mult)
                            abf = epool.tile([128, DH], bf16, tag='abf')
                            nc.vector.tensor_copy(out=abf[:], in_=actp[:])
                            nc.sync.dma_start(out=t2loc[gs, :], in_=abf[:])
                            if dbg is not None:
                                nc.sync.dma_start(out=dbg[gs, :], in_=actp[:])
                            trp = trps.tile([128, 128], f32, tag='tr')
                            nc.tensor.transpose(out=trp[:], in_=actp[:],
                                                identity=ident[:])
                            nc.vector.tensor_copy(out=t2t[:, gs], in_=trp[:])
                        else:
                            nc.vector.tensor_tensor(out=aggT[:], in0=psum[:],
                                                    in1=t2t[:, gs],
                                                    op=mybir.AluOpType.add)
                            up_t = ups.tile([128, DH], f32, tag='u')
                            nc.tensor.matmul(out=up_t[:, :DOUT], lhsT=aggT[:],
                                             rhs=w2_t[:], start=True, stop=True)
                            t1 = fpool.tile([128, DOUT], f32, tag='o1')
                            nc.vector.tensor_scalar(
                                out=t1[:], in0=up_t[:, :DOUT],
                                scalar1=dinv_t[:, g:g + 1], scalar2=None,
                                op0=mybir.AluOpType.mult)
                            t2 = fpool.tile([128, DOUT], f32, tag='o2')
                            nc.vector.tensor_tensor(out=t2[:], in0=t1[:],
                                                    in1=b2_t[:],
                                                    op=mybir.AluOpType.add)
                            mx = epool.tile([128, 1], f32, tag='mx')
                            nc.vector.reduce_max(out=mx[:], in_=t2[:],
                                                 axis=mybir.AxisListType.X)
                            nc.vector.tensor_scalar(
                                out=sh_all[:, g, :], in0=t2[:],
                                scalar1=mx[:, 0:1],
                                scalar2=None, op0=mybir.AluOpType.subtract)
                            ex = fpool.tile([128, DOUT], f32, tag='ex')
                            nc.scalar.activation(
                                out=ex[:], in_=sh_all[:, g, :],
                                func=mybir.ActivationFunctionType.Exp,
                                accum_out=se_all[:, g:g + 1])

            import os
            stage_env = os.environ.get('GNN_STAGE', 'full')
            do_layer(0)
            if stage_env in ('l1c', 'full'):
                nc.gpsimd.collective_compute(
                    'AllGather', mybir.AluOpType.bypass,
                    replica_groups=[list(range(NC))],
                    ins=[t2loc[:].opt()],
                    outs=[t2full[:].opt()])
            if stage_env == 'full':
                do_layer(1)
                ls_all = const.tile([128, G], f32)
                nc.scalar.activation(
                    out=ls_all[:], in_=se_all[:],
                    func=mybir.ActivationFunctionType.Ln)
                for g in range(G):
                    res = fpool.tile([128, DOUT], f32, tag='res')
                    nc.vector.tensor_scalar(
                        out=res[:], in0=sh_all[:, g, :],
                        scalar1=ls_all[:, g:g + 1],
                        scalar2=None, op0=mybir.AluOpType.subtract)
                    nc.sync.dma_start(out=out[g * 128:(g + 1) * 128, :],
                                      in_=res[:])
    nc.compile()
    return nc


def kernel(x, edge_index, W1, b1, W2, b2):
    x = np.asarray(x, dtype=np.float32)
    edge_index = np.asarray(edge_index)
    in_maps, ppos, meta = _preprocess(x, edge_index, np.asarray(W1),
                                      np.asarray(b1), np.asarray(W2),
                                      np.asarray(b2))
    key = (meta['TOT'], meta['KMAX'], meta['K_gb'].tobytes())
    if key not in _cache:
        _cache[key] = _build_nc(meta)
    nc = _cache[key]
    import os
    trace = bool(os.environ.get('GNN_TRACE'))
    r = run_bass_kernel_spmd(nc, in_maps, core_ids=list(range(NC)),
                             trace=trace)
    kernel.last_exec_ns = r.exec_time_ns
    kernel.last_results = r
    full = np.concatenate([r.results[c]['out'] for c in range(NC)], axis=0)
    return np.ascontiguousarray(full[ppos]).astype(np.float32)

